# revision 15
# baseline (speedup 1.0000x reference)
"""Cross-attention Trainium2 kernel (B=8, T=1024, S=1500, D=1024, H=16, Dh=64).

Sharding: pure data-parallel on batch — core b computes batch b end to end
(no collectives). Per-core pipeline, all operands SBUF-resident in bf16.

The kernel is ACT/PE co-bound: softmax needs exp of H*T*S scores on ScalarE
(128 lanes @ 1.2 GHz, ~10.3us/phase busy) while PE carries ~28.7k cycles of
matmul per phase (~12us at full clock). The schedule keeps BOTH saturated:

  - each head pair runs in two T-half phases; per phase the 24 score
    quarters [128s x 512t] pack HEAD-MAJOR into eight [128,1536] PSUM
    tiles: tiles 0-3 are head a's 12 s-chunks, tiles 4-7 head b's.
  - head-major packing lets pv0 (head a's PV accumulator) finish and
    evict MID-phase, freeing its PSUM ring slot for the next pair's
    q_proj burst mid-phase; its qTh evict lands before the boundary.
  - so the next phase's first TWO score tiles can be emitted at the
    boundary with zero serial work in front of them — the ACT stream
    never drains across a phase boundary.
  - out_proj drains per f-tile in the tail with store DMAs overlapped;
    PE stays continuously busy (holds the 2.4 GHz p-state).

Host side pre-transposes/casts inputs, pre-blocks wq/wo per out-tile, and
transposes the [f,t] output back.
"""

import sys

for _p in ("/opt/trn_rl_repo", "/root/.axon_site/_ro/trn_rl_repo"):
    if _p not in sys.path:
        sys.path.insert(0, _p)

import numpy as np
import ml_dtypes

import concourse.bass as bass
import concourse.mybir as mybir
import concourse.tile as tile
from concourse import bacc
from concourse import bass_utils

BF16 = ml_dtypes.bfloat16

P = 128
B = 8
T = 1024
S0 = 1500          # real source length
S = 1536           # padded to 12*128
D = 1024
H = 16
Dh = 64
DT = D // P        # 8 d/e/f tiles
ST = S // P        # 12 s chunks
NPAIR = H // 2     # 8 head pairs
HW = Dh + 1        # 65: per-head v width incl. ones column
SCALE = Dh ** -0.5
TH = 512           # T-half width (phase granularity)
NT = 8             # [128,1536] exp tiles per phase (3 quarters each)

f32 = mybir.dt.float32
bf16 = mybir.dt.bfloat16


def build_bass():
    nc = bacc.Bacc("TRN2", target_bir_lowering=False, debug=False,
                   enable_asserts=False, num_devices=B)

    xT_d = nc.dram_tensor("xT", [D, T], bf16, kind="ExternalInput")
    kT_d = nc.dram_tensor("kT", [D, S], bf16, kind="ExternalInput")
    va_d = nc.dram_tensor("vaug", [S, H * HW], bf16, kind="ExternalInput")
    # wqb/wob pre-blocked: rows j*128.. hold the eight [128,128] lhsT slabs
    # of out-tile j, so one [128,1024] DMA covers e/f-tile j.
    wqb_d = nc.dram_tensor("wqb", [D, D], bf16, kind="ExternalInput")
    bq_d = nc.dram_tensor("bqr", [P, DT], f32, kind="ExternalInput")
    wob_d = nc.dram_tensor("wob", [D, D], bf16, kind="ExternalInput")
    bo_d = nc.dram_tensor("bor", [P, DT], f32, kind="ExternalInput")
    outT_d = nc.dram_tensor("outT", [D, T], f32, kind="ExternalOutput")

    EXP = mybir.ActivationFunctionType.Exp

    with tile.TileContext(nc) as tc:
        with (
            tc.tile_pool(name="const", bufs=1) as cp,
            tc.tile_pool(name="work", bufs=2) as wp,
            tc.tile_pool(name="psum_mm", bufs=2, space="PSUM") as mmp,
            tc.tile_pool(name="psum_pv", bufs=2, space="PSUM") as pvp,
        ):
            def load1(dram, cols, j, tagbase, dt=bf16):
                t = cp.tile([P, cols], dt, name=f"{tagbase}{j}",
                            tag=f"{tagbase}{j}")
                nc.sync.dma_start(t[:], dram[j * P:(j + 1) * P, :])
                return t

            # warm the ACT exp table while DMAs stream
            dummy = cp.tile([1, 8], f32, name="dummy", tag="dummy")
            nc.vector.memset(dummy[:], 0.0)
            nc.scalar.activation(dummy[:], dummy[:], EXP)

            # warm the PE p-state during the input DMA wait: ~5us of junk
            # matmuls so the real q_proj burst starts at full clock
            wrm = cp.tile([P, TH], bf16, name="wrm", tag="wrm")
            nc.vector.memset(wrm[:], 0.0)
            wps = mmp.tile([P, TH], f32, name="wps", tag="mm")
            for _ in range(22):
                nc.tensor.matmul(wps[:, :], lhsT=wrm[:, 0:P], rhs=wrm[:, :],
                                 start=True, stop=True)

            # xT as per-half tiles: q_proj phase ph reads only half ph, and
            # separate tiles avoid coarse per-tile false deps on the DMAs.
            def loadx(dt_i, ph):
                t = cp.tile([P, TH], bf16, name=f"xTs{dt_i}_{ph}",
                            tag=f"xTs{dt_i}_{ph}")
                nc.sync.dma_start(
                    t[:], xT_d[dt_i * P:(dt_i + 1) * P,
                               ph * TH:(ph + 1) * TH])
                return t

            # DMA priority order: phase (0,0)'s critical path first — q_proj
            # inputs, then kT[0] + ALL va chunks (pv0 consumes every chunk
            # within the first half-phase), then half-1 inputs, then the
            # rest in pair order.
            wqb_sb = {0: load1(wqb_d, D, 0, "wqbs")}
            xTh_sb = {(dt_i, 0): loadx(dt_i, 0) for dt_i in range(DT)}
            bq_sb = cp.tile([P, DT], f32, name="bq_sb", tag="bq_sb")
            nc.sync.dma_start(bq_sb[:], bq_d[:, :])
            kT_sb = {0: load1(kT_d, S, 0, "kTs")}
            va_sb = {c: load1(va_d, H * HW, c, "vas") for c in range(6)}
            xTh_sb.update({(dt_i, 1): loadx(dt_i, 1) for dt_i in range(DT)})
            va_sb.update({c: load1(va_d, H * HW, c, "vas")
                          for c in range(6, ST)})
            kT_sb[1] = load1(kT_d, S, 1, "kTs")
            wqb_sb[1] = load1(wqb_d, D, 1, "wqbs")
            for j in range(2, DT):
                kT_sb[j] = load1(kT_d, S, j, "kTs")
                wqb_sb[j] = load1(wqb_d, D, j, "wqbs")
            wob_sb = [load1(wob_d, D, j, "wobs") for j in range(DT)]
            bo_sb = cp.tile([P, DT], f32, name="bo_sb", tag="bo_sb")
            nc.sync.dma_start(bo_sb[:], bo_d[:, :])

            # qT/aT as per-half tiles: the q_proj evict writing half ph must
            # not serialize score matmuls reading the other half.
            qTh_sb = {(j, ph): cp.tile([P, TH], bf16, name=f"qTs{j}_{ph}",
                                       tag=f"qTs{j}_{ph}")
                      for j in range(DT) for ph in range(2)}
            aTh_sb = {(j, ph): cp.tile([P, TH], bf16, name=f"aTs{j}_{ph}",
                                       tag=f"aTs{j}_{ph}")
                      for j in range(DT) for ph in range(2)}

            # ---- q projection for e-tile j, one T-half (8 MMs + evict) ----
            # qp lives in an MM-ring slot (its [128,512] fits the 3-bank
            # slot), so the burst can run at phase START where PE would
            # otherwise idle waiting on the first exp sems.
            def qproj_burst(j, ph):
                qp = mmp.tile([P, TH], f32, name=f"qp{j}_{ph}", tag="mm")
                for dt_i in range(DT):
                    nc.tensor.matmul(
                        qp[:, :],
                        lhsT=wqb_sb[j][:, dt_i * P:(dt_i + 1) * P],
                        rhs=xTh_sb[(dt_i, ph)][:, :],
                        start=(dt_i == 0), stop=(dt_i == DT - 1),
                    )
                nc.vector.tensor_scalar_add(qTh_sb[(j, ph)][:, :],
                                            qp[:, :], bq_sb[:, j:j + 1])

            # prologue: qT[0,0] only — (0,1)'s burst runs inside phase (0,0)
            qproj_burst(0, 0)

            # ---- attention: pair j, phase ph covers t-half ph --------------
            # Exp tile k holds HEAD-MAJOR quarters: tile k quarter q is
            # (head a = k//4, chunk c = 3*(k%4)+q).
            def emit_sc_exp(j, ph, k):
                sct = mmp.tile([P, 3 * TH], f32, name=f"sc{j}{ph}{k}",
                               tag="mm")
                a = k // 4
                rows = slice(a * Dh, (a + 1) * Dh)
                for q in range(3):
                    c = 3 * (k % 4) + q
                    nc.tensor.matmul(
                        sct[:, q * TH:(q + 1) * TH],
                        lhsT=kT_sb[j][rows, c * P:(c + 1) * P],
                        rhs=qTh_sb[(j, ph)][rows, :],
                        start=True, stop=True,
                    )
                pt = wp.tile([P, 3 * TH], bf16, name=f"pt{j}{ph}{k}",
                             tag="pt", bufs=8)
                nc.scalar.activation(pt[:, :], sct[:, :], EXP)
                return pt

            def emit_pv(j, ph, k, pt, pv):
                a = k // 4
                h = 2 * j + a
                for q in range(3):
                    c = 3 * (k % 4) + q
                    nc.tensor.matmul(
                        pv[a][0:HW, :],
                        lhsT=va_sb[c][:, h * HW:(h + 1) * HW],
                        rhs=pt[:, q * TH:(q + 1) * TH],
                        start=(c == 0), stop=(c == ST - 1),
                    )

            def evict_pv(j, ph, a, pv, on_act=False):
                sb = wp.tile([HW, TH], f32, name=f"pvsb{j}{ph}{a}",
                             tag="pvsb", bufs=4)
                if on_act:
                    nc.scalar.copy(sb[:, :], pv[a][0:HW, :])
                else:
                    nc.vector.tensor_copy(sb[:, :], pv[a][0:HW, :])
                return sb

            def normalize(j, ph, a, sb):
                # attnT = pv[0:64] / pv[64] (DVE + gpsimd)
                dsm = wp.tile([Dh, TH // Dh], f32,
                              name=f"ds{j}{ph}{a}", tag="dsm", bufs=4)
                nc.sync.dma_start(dsm[:, :], sb[Dh:Dh + 1, :])
                nc.vector.reciprocal(dsm[:, :], dsm[:, :])
                rrow = wp.tile([1, TH], f32, name=f"rr{j}{ph}{a}",
                               tag="rrow", bufs=4)
                nc.sync.dma_start(rrow[:, :], dsm[:, :])
                nrm = wp.tile([Dh, TH], f32, name=f"nr{j}{ph}{a}",
                              tag="nrm", bufs=4)
                nc.gpsimd.partition_broadcast(nrm[:, :], rrow[0:1, :])
                nc.vector.tensor_mul(
                    aTh_sb[(j, ph)][a * Dh:(a + 1) * Dh, :],
                    sb[0:Dh, :], nrm[:, :])

            # Per phase: tiles 0-3 (head a) stream first; after tile 3's PV,
            # head a's accumulator evicts mid-phase, freeing its ring slot
            # for the next pair's q_proj burst; tiles 4-7 (head b) stream
            # over it.  At the boundary only pv1 evicts, and the next
            # phase's first two score tiles (pend) go out with nothing
            # serial in front of them, so ACT never drains.
            pend = {}
            pend[0] = emit_sc_exp(0, 0, 0)
            pend[1] = emit_sc_exp(0, 0, 1)
            for j in range(NPAIR):
                for ph in range(2):
                    last = (j, ph) == (NPAIR - 1, 1)
                    if not last:
                        nj, nph = (j, 1) if ph == 0 else (j + 1, 0)
                    pv = [pvp.tile([P, TH], f32, name=f"pv{j}_{ph}_{a}",
                                   tag="pv") for a in range(2)]
                    pt = {0: pend.pop(0), 1: pend.pop(1)}
                    # next pair's q_proj at phase start (PE would otherwise
                    # idle on the first exp sems); its qTh evict lands well
                    # before the mm-ring slot is recycled by sc3.
                    if not last:
                        qproj_burst(nj, nph)
                    # strict lag-1 interleave: sc(k) paces on exp(k-2)'s
                    # slot release, pv(k-2) on exp(k-2)'s output — both
                    # sems pre-fired, so PE never stalls mid-phase.
                    pt[2] = emit_sc_exp(j, ph, 2)
                    emit_pv(j, ph, 0, pt[0], pv)
                    pt[3] = emit_sc_exp(j, ph, 3)
                    emit_pv(j, ph, 1, pt[1], pv)
                    pt[4] = emit_sc_exp(j, ph, 4)
                    emit_pv(j, ph, 2, pt[2], pv)
                    pt[5] = emit_sc_exp(j, ph, 5)
                    emit_pv(j, ph, 3, pt[3], pv)
                    # head a complete
                    sb0 = evict_pv(j, ph, 0, pv)
                    normalize(j, ph, 0, sb0)
                    pt[6] = emit_sc_exp(j, ph, 6)
                    emit_pv(j, ph, 4, pt[4], pv)
                    pt[7] = emit_sc_exp(j, ph, 7)
                    emit_pv(j, ph, 5, pt[5], pv)
                    if not last:
                        pend[0] = emit_sc_exp(nj, nph, 0)
                    emit_pv(j, ph, 6, pt[6], pv)
                    if not last:
                        pend[1] = emit_sc_exp(nj, nph, 1)
                    emit_pv(j, ph, 7, pt[7], pv)
                    sb1 = evict_pv(j, ph, 1, pv, on_act=last)
                    normalize(j, ph, 1, sb1)

            # ---- out projection  outT[f,t] ---------------------------------
            for fj in range(DT):
                ps = mmp.tile([P, T], f32, name=f"op{fj}", tag="mm")
                for tch in range(2):
                    tsl = slice(tch * TH, (tch + 1) * TH)
                    for et in range(DT):
                        nc.tensor.matmul(
                            ps[:, tsl],
                            lhsT=wob_sb[fj][:, et * P:(et + 1) * P],
                            rhs=aTh_sb[(et, tch)][:, :],
                            start=(et == 0), stop=(et == DT - 1),
                        )
                for tch in range(2):
                    tsl = slice(tch * TH, (tch + 1) * TH)
                    ost = wp.tile([P, TH], f32, name=f"ost{fj}_{tch}",
                                  tag="ost", bufs=3)
                    nc.vector.tensor_scalar_add(ost[:, :], ps[:, tsl],
                                                bo_sb[:, fj:fj + 1])
                    nc.sync.dma_start(outT_d[fj * P:(fj + 1) * P, tsl],
                                      ost[:, :])

    nc.compile()
    return nc


def prep_inputs(x, k, v, wq, bq, wo, bo):
    """Host-side shard + layout prep. Returns per-core in_maps."""
    x = np.asarray(x, np.float32)
    k = np.asarray(k, np.float32)
    v = np.asarray(v, np.float32)
    wq = np.asarray(wq, np.float32)
    bq = np.asarray(bq, np.float32)
    wo = np.asarray(wo, np.float32)
    bo = np.asarray(bo, np.float32)

    wqT = np.ascontiguousarray((wq * SCALE).T).astype(BF16)       # [d, e]
    woT = np.ascontiguousarray(wo.T).astype(BF16)                 # [e, f]
    wqb = np.zeros((D, D), BF16)
    wob = np.zeros((D, D), BF16)
    for j in range(DT):
        for dt_i in range(DT):
            wqb[j * P:(j + 1) * P, dt_i * P:(dt_i + 1) * P] = \
                wqT[dt_i * P:(dt_i + 1) * P, j * P:(j + 1) * P]
            wob[j * P:(j + 1) * P, dt_i * P:(dt_i + 1) * P] = \
                woT[dt_i * P:(dt_i + 1) * P, j * P:(j + 1) * P]
    bqr = np.ascontiguousarray((bq * SCALE).reshape(DT, P).T)     # [P, DT]
    bor = np.ascontiguousarray(bo.reshape(DT, P).T)               # [P, DT]

    in_maps = []
    for b in range(x.shape[0]):
        xT = np.ascontiguousarray(x[b].T).astype(BF16)            # [D, T]
        kT = np.zeros((D, S), BF16)
        kT[:, :S0] = k[b].T.astype(BF16)
        vaug = np.zeros((S, H * HW), BF16)
        vb = v[b].astype(BF16)
        for h in range(H):
            vaug[:S0, h * HW:h * HW + Dh] = vb[:, h * Dh:(h + 1) * Dh]
            vaug[:S0, h * HW + Dh] = BF16(1.0)
        in_maps.append({
            "xT": xT, "kT": kT, "vaug": np.ascontiguousarray(vaug),
            "wqb": wqb, "bqr": bqr, "wob": wob, "bor": bor,
        })
    return in_maps


_NC_CACHE = {}


def kernel(x, k, v, wq, bq, wo, bo, _trace=False):
    if "nc" not in _NC_CACHE:
        _NC_CACHE["nc"] = build_bass()
    nc = _NC_CACHE["nc"]
    in_maps = prep_inputs(x, k, v, wq, bq, wo, bo)
    res = bass_utils.run_bass_kernel_spmd(
        nc, in_maps, core_ids=list(range(B)), trace=_trace)
    _NC_CACHE["last_result"] = res
    out = np.stack([np.ascontiguousarray(r["outT"].T) for r in res.results])
    return out


# revision 16
# speedup vs baseline: 1.0356x; 1.0356x over previous
"""Cross-attention Trainium2 kernel (B=8, T=1024, S=1500, D=1024, H=16, Dh=64).

Sharding: pure data-parallel on batch — core b computes batch b end to end
(no collectives). Per-core pipeline, all operands SBUF-resident in bf16.

The kernel is ACT/PE co-bound: softmax needs exp of H*T*S scores on ScalarE
(128 lanes @ 1.2 GHz, ~10.3us/phase busy) while PE carries ~28.7k cycles of
matmul per phase (~12us at full clock). The schedule keeps BOTH saturated:

  - each head pair runs in two T-half phases; per phase the 24 score
    quarters [128s x 512t] pack HEAD-MAJOR into eight [128,1536] PSUM
    tiles: tiles 0-3 are head a's 12 s-chunks, tiles 4-7 head b's.
  - head-major packing lets pv0 (head a's PV accumulator) finish and
    evict MID-phase, freeing its PSUM ring slot for the next pair's
    q_proj burst mid-phase; its qTh evict lands before the boundary.
  - so the next phase's first TWO score tiles can be emitted at the
    boundary with zero serial work in front of them — the ACT stream
    never drains across a phase boundary.
  - out_proj drains per f-tile in the tail with store DMAs overlapped;
    PE stays continuously busy (holds the 2.4 GHz p-state).

Host side pre-transposes/casts inputs, pre-blocks wq/wo per out-tile, and
transposes the [f,t] output back.
"""

import sys

for _p in ("/opt/trn_rl_repo", "/root/.axon_site/_ro/trn_rl_repo"):
    if _p not in sys.path:
        sys.path.insert(0, _p)

import numpy as np
import ml_dtypes

import concourse.bass as bass
import concourse.mybir as mybir
import concourse.tile as tile
from concourse import bacc
from concourse import bass_utils

BF16 = ml_dtypes.bfloat16

P = 128
B = 8
T = 1024
S0 = 1500          # real source length
S = 1536           # padded to 12*128
D = 1024
H = 16
Dh = 64
DT = D // P        # 8 d/e/f tiles
ST = S // P        # 12 s chunks
NPAIR = H // 2     # 8 head pairs
HW = Dh + 1        # 65: per-head v width incl. ones column
SCALE = Dh ** -0.5
TH = 512           # T-half width (phase granularity)
NT = 8             # [128,1536] exp tiles per phase (3 quarters each)

f32 = mybir.dt.float32
bf16 = mybir.dt.bfloat16


def build_bass():
    nc = bacc.Bacc("TRN2", target_bir_lowering=False, debug=False,
                   enable_asserts=False, num_devices=B)

    xT_d = nc.dram_tensor("xT", [D, T], bf16, kind="ExternalInput")
    kT_d = nc.dram_tensor("kT", [D, S], bf16, kind="ExternalInput")
    va_d = nc.dram_tensor("vaug", [S, H * HW], bf16, kind="ExternalInput")
    # wqb/wob pre-blocked: rows j*128.. hold the eight [128,128] lhsT slabs
    # of out-tile j, so one [128,1024] DMA covers e/f-tile j.
    wqb_d = nc.dram_tensor("wqb", [D, D], bf16, kind="ExternalInput")
    bq_d = nc.dram_tensor("bqr", [P, DT], f32, kind="ExternalInput")
    wob_d = nc.dram_tensor("wob", [D, D], bf16, kind="ExternalInput")
    bo_d = nc.dram_tensor("bor", [P, DT], f32, kind="ExternalInput")
    outT_d = nc.dram_tensor("outT", [D, T], f32, kind="ExternalOutput")

    EXP = mybir.ActivationFunctionType.Exp

    with tile.TileContext(nc) as tc:
        with (
            tc.tile_pool(name="const", bufs=1) as cp,
            tc.tile_pool(name="work", bufs=2) as wp,
            tc.tile_pool(name="psum_mm", bufs=2, space="PSUM") as mmp,
            tc.tile_pool(name="psum_pv", bufs=2, space="PSUM") as pvp,
        ):
            def load1(dram, cols, j, tagbase, dt=bf16):
                t = cp.tile([P, cols], dt, name=f"{tagbase}{j}",
                            tag=f"{tagbase}{j}")
                nc.sync.dma_start(t[:], dram[j * P:(j + 1) * P, :])
                return t

            # warm the ACT exp table while DMAs stream
            dummy = cp.tile([1, 8], f32, name="dummy", tag="dummy")
            nc.vector.memset(dummy[:], 0.0)
            nc.scalar.activation(dummy[:], dummy[:], EXP)

            # warm the PE p-state during the input DMA wait: ~5us of junk
            # matmuls so the real q_proj burst starts at full clock
            wrm = cp.tile([P, TH], bf16, name="wrm", tag="wrm")
            nc.vector.memset(wrm[:], 0.0)
            wps = mmp.tile([P, TH], f32, name="wps", tag="mm")
            for _ in range(22):
                nc.tensor.matmul(wps[:, :], lhsT=wrm[:, 0:P], rhs=wrm[:, :],
                                 start=True, stop=True)

            # xT as per-half tiles: q_proj phase ph reads only half ph, and
            # separate tiles avoid coarse per-tile false deps on the DMAs.
            def loadx(dt_i, ph):
                t = cp.tile([P, TH], bf16, name=f"xTs{dt_i}_{ph}",
                            tag=f"xTs{dt_i}_{ph}")
                nc.sync.dma_start(
                    t[:], xT_d[dt_i * P:(dt_i + 1) * P,
                               ph * TH:(ph + 1) * TH])
                return t

            # DMA priority order: phase (0,0)'s critical path first — q_proj
            # inputs, then kT[0] + ALL va chunks (pv0 consumes every chunk
            # within the first half-phase), then half-1 inputs, then the
            # rest in pair order.
            wqb_sb = {0: load1(wqb_d, D, 0, "wqbs")}
            xTh_sb = {(dt_i, 0): loadx(dt_i, 0) for dt_i in range(DT)}
            bq_sb = cp.tile([P, DT], f32, name="bq_sb", tag="bq_sb")
            nc.sync.dma_start(bq_sb[:], bq_d[:, :])
            kT_sb = {0: load1(kT_d, S, 0, "kTs")}
            # xT half-1 right away: phase (0,0) STARTS with q_proj(0,1)
            # in the mm ring — a late xT1 stalls the whole in-order PE
            # stream of the first phase.
            xTh_sb.update({(dt_i, 1): loadx(dt_i, 1) for dt_i in range(DT)})
            va_sb = {c: load1(va_d, H * HW, c, "vas") for c in range(ST)}
            kT_sb[1] = load1(kT_d, S, 1, "kTs")
            wqb_sb[1] = load1(wqb_d, D, 1, "wqbs")
            for j in range(2, DT):
                kT_sb[j] = load1(kT_d, S, j, "kTs")
                wqb_sb[j] = load1(wqb_d, D, j, "wqbs")
            wob_sb = [load1(wob_d, D, j, "wobs") for j in range(DT)]
            bo_sb = cp.tile([P, DT], f32, name="bo_sb", tag="bo_sb")
            nc.sync.dma_start(bo_sb[:], bo_d[:, :])

            # qT/aT as per-half tiles: the q_proj evict writing half ph must
            # not serialize score matmuls reading the other half.
            qTh_sb = {(j, ph): cp.tile([P, TH], bf16, name=f"qTs{j}_{ph}",
                                       tag=f"qTs{j}_{ph}")
                      for j in range(DT) for ph in range(2)}
            aTh_sb = {(j, ph): cp.tile([P, TH], bf16, name=f"aTs{j}_{ph}",
                                       tag=f"aTs{j}_{ph}")
                      for j in range(DT) for ph in range(2)}

            # ---- q projection for e-tile j, one T-half (8 MMs + evict) ----
            # qp lives in an MM-ring slot (its [128,512] fits the 3-bank
            # slot), so the burst can run at phase START where PE would
            # otherwise idle waiting on the first exp sems.
            def qproj_burst(j, ph):
                qp = mmp.tile([P, TH], f32, name=f"qp{j}_{ph}", tag="mm")
                for dt_i in range(DT):
                    nc.tensor.matmul(
                        qp[:, :],
                        lhsT=wqb_sb[j][:, dt_i * P:(dt_i + 1) * P],
                        rhs=xTh_sb[(dt_i, ph)][:, :],
                        start=(dt_i == 0), stop=(dt_i == DT - 1),
                    )
                nc.vector.tensor_scalar_add(qTh_sb[(j, ph)][:, :],
                                            qp[:, :], bq_sb[:, j:j + 1])

            # prologue: qT[0,0] only — (0,1)'s burst runs inside phase (0,0)
            qproj_burst(0, 0)

            # ---- attention: pair j, phase ph covers t-half ph --------------
            # Exp tile k holds HEAD-MAJOR quarters: tile k quarter q is
            # (head a = k//4, chunk c = 3*(k%4)+q).
            def emit_sc_exp(j, ph, k):
                sct = mmp.tile([P, 3 * TH], f32, name=f"sc{j}{ph}{k}",
                               tag="mm")
                a = k // 4
                rows = slice(a * Dh, (a + 1) * Dh)
                for q in range(3):
                    c = 3 * (k % 4) + q
                    nc.tensor.matmul(
                        sct[:, q * TH:(q + 1) * TH],
                        lhsT=kT_sb[j][rows, c * P:(c + 1) * P],
                        rhs=qTh_sb[(j, ph)][rows, :],
                        start=True, stop=True,
                    )
                pt = wp.tile([P, 3 * TH], bf16, name=f"pt{j}{ph}{k}",
                             tag="pt", bufs=8)
                nc.scalar.activation(pt[:, :], sct[:, :], EXP)
                return pt

            def emit_pv(j, ph, k, pt, pv):
                a = k // 4
                h = 2 * j + a
                for q in range(3):
                    c = 3 * (k % 4) + q
                    nc.tensor.matmul(
                        pv[a][0:HW, :],
                        lhsT=va_sb[c][:, h * HW:(h + 1) * HW],
                        rhs=pt[:, q * TH:(q + 1) * TH],
                        start=(c == 0), stop=(c == ST - 1),
                    )

            def evict_pv(j, ph, a, pv, on_act=False):
                sb = wp.tile([HW, TH], f32, name=f"pvsb{j}{ph}{a}",
                             tag="pvsb", bufs=4)
                if on_act:
                    nc.scalar.copy(sb[:, :], pv[a][0:HW, :])
                else:
                    nc.vector.tensor_copy(sb[:, :], pv[a][0:HW, :])
                return sb

            def normalize(j, ph, a, sb):
                # attnT = pv[0:64] / pv[64] (DVE + gpsimd)
                dsm = wp.tile([Dh, TH // Dh], f32,
                              name=f"ds{j}{ph}{a}", tag="dsm", bufs=4)
                nc.sync.dma_start(dsm[:, :], sb[Dh:Dh + 1, :])
                nc.vector.reciprocal(dsm[:, :], dsm[:, :])
                rrow = wp.tile([1, TH], f32, name=f"rr{j}{ph}{a}",
                               tag="rrow", bufs=4)
                nc.sync.dma_start(rrow[:, :], dsm[:, :])
                nrm = wp.tile([Dh, TH], f32, name=f"nr{j}{ph}{a}",
                              tag="nrm", bufs=4)
                nc.gpsimd.partition_broadcast(nrm[:, :], rrow[0:1, :])
                nc.vector.tensor_mul(
                    aTh_sb[(j, ph)][a * Dh:(a + 1) * Dh, :],
                    sb[0:Dh, :], nrm[:, :])

            # Per phase: tiles 0-3 (head a) stream first; after tile 3's PV,
            # head a's accumulator evicts mid-phase, freeing its ring slot
            # for the next pair's q_proj burst; tiles 4-7 (head b) stream
            # over it.  At the boundary only pv1 evicts, and the next
            # phase's first two score tiles (pend) go out with nothing
            # serial in front of them, so ACT never drains.
            pend = {}
            pend[0] = emit_sc_exp(0, 0, 0)
            pend[1] = emit_sc_exp(0, 0, 1)
            for j in range(NPAIR):
                for ph in range(2):
                    last = (j, ph) == (NPAIR - 1, 1)
                    if not last:
                        nj, nph = (j, 1) if ph == 0 else (j + 1, 0)
                    pv = [pvp.tile([P, TH], f32, name=f"pv{j}_{ph}_{a}",
                                   tag="pv") for a in range(2)]
                    pt = {0: pend.pop(0), 1: pend.pop(1)}
                    # next pair's q_proj at phase start (PE would otherwise
                    # idle on the first exp sems); its qTh evict lands well
                    # before the mm-ring slot is recycled by sc3.
                    if not last:
                        qproj_burst(nj, nph)
                    # strict lag-1 interleave: sc(k) paces on exp(k-2)'s
                    # slot release, pv(k-2) on exp(k-2)'s output — both
                    # sems pre-fired, so PE never stalls mid-phase.
                    pt[2] = emit_sc_exp(j, ph, 2)
                    emit_pv(j, ph, 0, pt[0], pv)
                    pt[3] = emit_sc_exp(j, ph, 3)
                    emit_pv(j, ph, 1, pt[1], pv)
                    pt[4] = emit_sc_exp(j, ph, 4)
                    emit_pv(j, ph, 2, pt[2], pv)
                    pt[5] = emit_sc_exp(j, ph, 5)
                    emit_pv(j, ph, 3, pt[3], pv)
                    # head a complete
                    sb0 = evict_pv(j, ph, 0, pv)
                    normalize(j, ph, 0, sb0)
                    pt[6] = emit_sc_exp(j, ph, 6)
                    emit_pv(j, ph, 4, pt[4], pv)
                    pt[7] = emit_sc_exp(j, ph, 7)
                    emit_pv(j, ph, 5, pt[5], pv)
                    if not last:
                        pend[0] = emit_sc_exp(nj, nph, 0)
                    emit_pv(j, ph, 6, pt[6], pv)
                    if not last:
                        pend[1] = emit_sc_exp(nj, nph, 1)
                    emit_pv(j, ph, 7, pt[7], pv)
                    sb1 = evict_pv(j, ph, 1, pv, on_act=last)
                    normalize(j, ph, 1, sb1)

            # ---- out projection  outT[f,t] ---------------------------------
            for fj in range(DT):
                ps = mmp.tile([P, T], f32, name=f"op{fj}", tag="mm")
                for tch in range(2):
                    tsl = slice(tch * TH, (tch + 1) * TH)
                    for et in range(DT):
                        nc.tensor.matmul(
                            ps[:, tsl],
                            lhsT=wob_sb[fj][:, et * P:(et + 1) * P],
                            rhs=aTh_sb[(et, tch)][:, :],
                            start=(et == 0), stop=(et == DT - 1),
                        )
                for tch in range(2):
                    tsl = slice(tch * TH, (tch + 1) * TH)
                    ost = wp.tile([P, TH], f32, name=f"ost{fj}_{tch}",
                                  tag="ost", bufs=3)
                    nc.vector.tensor_scalar_add(ost[:, :], ps[:, tsl],
                                                bo_sb[:, fj:fj + 1])
                    nc.sync.dma_start(outT_d[fj * P:(fj + 1) * P, tsl],
                                      ost[:, :])

    nc.compile()
    return nc


def prep_inputs(x, k, v, wq, bq, wo, bo):
    """Host-side shard + layout prep. Returns per-core in_maps."""
    x = np.asarray(x, np.float32)
    k = np.asarray(k, np.float32)
    v = np.asarray(v, np.float32)
    wq = np.asarray(wq, np.float32)
    bq = np.asarray(bq, np.float32)
    wo = np.asarray(wo, np.float32)
    bo = np.asarray(bo, np.float32)

    wqT = np.ascontiguousarray((wq * SCALE).T).astype(BF16)       # [d, e]
    woT = np.ascontiguousarray(wo.T).astype(BF16)                 # [e, f]
    wqb = np.zeros((D, D), BF16)
    wob = np.zeros((D, D), BF16)
    for j in range(DT):
        for dt_i in range(DT):
            wqb[j * P:(j + 1) * P, dt_i * P:(dt_i + 1) * P] = \
                wqT[dt_i * P:(dt_i + 1) * P, j * P:(j + 1) * P]
            wob[j * P:(j + 1) * P, dt_i * P:(dt_i + 1) * P] = \
                woT[dt_i * P:(dt_i + 1) * P, j * P:(j + 1) * P]
    bqr = np.ascontiguousarray((bq * SCALE).reshape(DT, P).T)     # [P, DT]
    bor = np.ascontiguousarray(bo.reshape(DT, P).T)               # [P, DT]

    in_maps = []
    for b in range(x.shape[0]):
        xT = np.ascontiguousarray(x[b].T).astype(BF16)            # [D, T]
        kT = np.zeros((D, S), BF16)
        kT[:, :S0] = k[b].T.astype(BF16)
        vaug = np.zeros((S, H * HW), BF16)
        vb = v[b].astype(BF16)
        for h in range(H):
            vaug[:S0, h * HW:h * HW + Dh] = vb[:, h * Dh:(h + 1) * Dh]
            vaug[:S0, h * HW + Dh] = BF16(1.0)
        in_maps.append({
            "xT": xT, "kT": kT, "vaug": np.ascontiguousarray(vaug),
            "wqb": wqb, "bqr": bqr, "wob": wob, "bor": bor,
        })
    return in_maps


_NC_CACHE = {}


def kernel(x, k, v, wq, bq, wo, bo, _trace=False):
    if "nc" not in _NC_CACHE:
        _NC_CACHE["nc"] = build_bass()
    nc = _NC_CACHE["nc"]
    in_maps = prep_inputs(x, k, v, wq, bq, wo, bo)
    res = bass_utils.run_bass_kernel_spmd(
        nc, in_maps, core_ids=list(range(B)), trace=_trace)
    _NC_CACHE["last_result"] = res
    out = np.stack([np.ascontiguousarray(r["outT"].T) for r in res.results])
    return out
